# revision 11
# baseline (speedup 1.0000x reference)
"""Trainium2 Bass kernel for nn_InteractionBlock (gnn_message_passing).

Algebraic transformation: per angle alpha with (s, t) = (src, tgt):
    sm[alpha] = (msg[s] @ Ws + bs) * d[t]
    out[alpha] = sum_b a[t, b] * (Wb[:, b, :] @ sm[alpha])
    agg[t] = sum_{alpha: tgt=t} out[alpha]
Everything except msg[s] depends only on t, so with
    S[t] = sum_{alpha: tgt=t} msg[s(alpha)]   and  c[t] = |{alpha: tgt=t}|
    agg[t] = sum_b a[t,b] * (Wb[:,b,:] @ ((S[t] @ Ws + c[t]*bs) * d[t]))
The A=600K gather+einsum+scatter collapses to an E-sized dense pipeline
after a segment-sum of raw msg rows.

v2 design: the segment-sum S is computed on the HOST (one scipy sparse
matmul, ~0.1s), so every core's work is fully local: no collectives, no
indirect DMA, no data-dependent slot tables. Each core receives a single
blob with its own 12.5K-edge slice (msg^T, S^T, x_dist^T, a/cnt rows,
replicated weights) and runs a fixed dense per-edge pipeline. With no
cross-core dependency, a slow or wedged host->device stream on one core
cannot stall the other seven inside a collective, and the BIR is fully
static, so a prebuilt module + NEFF are embedded below and the in-kernel
build/compile steps collapse to a cache load. The device returns
delta = out - msg in fp8 (|delta| ~ 0.2|out|); the host adds msg back in
f32.
"""

import base64
import os
import sys
import time
import zlib

import numpy as np

sys.path.insert(0, "/opt/trn_rl_repo")

# heavy imports at module scope (outside the kernel() hot path)
import ml_dtypes  # noqa: E402
import jax  # noqa: E402
from jax.sharding import Mesh, NamedSharding, PartitionSpec  # noqa: E402
import concourse.tile as tile  # noqa: E402
import concourse.bass as bass  # noqa: E402
from concourse import bacc, mybir  # noqa: E402
from concourse import bass2jax as _b2j  # noqa: E402

try:  # kick off backend/device discovery + first-touch link warmup early
    _DEVICES = jax.devices()
    _WARM = jax.device_put(
        np.zeros((len(_DEVICES), 8), np.float32),
        NamedSharding(Mesh(np.asarray(_DEVICES), ("core",)),
                      PartitionSpec("core")))
except Exception:
    _DEVICES = None
    _WARM = None

_T0 = None


def _tick(label):
    global _T0
    if os.environ.get("KTIME"):
        now = time.time()
        if _T0 is None:
            _T0 = now
        print(f"[ktime] {now - _T0:7.2f}s  {label}", file=sys.stderr,
              flush=True)

E = 100000
NR = 6
NS = 7
H = 128
BD = 8
M = 128
P = 8           # cores
ES = E // P     # 12500 edges per core
NT = 512        # dense-phase column tile
NSP = 12800     # padded edges per core (25 * 512)
NTILES = NSP // NT  # 25

# packed bf16 weight slab: name -> (col offset, rows, cols)
_WOFF = {}
_c = 0
for _n, _r, _k in [("Wd", NR, H), ("Ws", M, H), ("bs_row", 1, H),
                   ("Wt", M, H), ("bt_row", 1, H), ("WbT", H, BD * H),
                   ("rb_w1", H, H), ("rb_w2", H, H), ("Wskip", H, M),
                   ("ra1_w1", M, M), ("ra1_w2", M, M),
                   ("ra2_w1", M, M), ("ra2_w2", M, M)]:
    _WOFF[_n] = (_c, _r, _k)
    _c += _k
WCOLS = _c  # 2560
_BIAS_NAMES = ["rb_b1", "rb_b2", "bskip", "ra1_b1", "ra1_b2",
               "ra2_b1", "ra2_b2"]

# byte layout of the single per-core operand blob; all sections 4B-aligned
_LAYOUT = {}
_off = 0
for _name, _dt, _esize, _rows, _cols in [
        ("BIA", "f32", 4, 128, len(_BIAS_NAMES)),
        ("WB16", "bf16", 2, 128, WCOLS),
        ("xdTc", "bf16", 2, NR, NSP),
        ("acc", "bf16", 2, 1, NTILES * (BD + 1) * NT),
        ("msgT", "bf16", 2, M, NSP),
        ("ST", "bf16", 2, M, NSP)]:
    _nb = _esize * _rows * _cols
    assert _nb % 4 == 0
    _LAYOUT[_name] = (_off, _nb, _dt, _rows, _cols)
    _off += _nb
NB = _off


def _bf16():
    return np.dtype(ml_dtypes.bfloat16)


_NEFF_CACHE_DIR = "/tmp/bass_neff_cache"

# Embedded prebuilt artifacts (zlib+base85). _EMBED_BIR is the serialized
# BIR module; _EMBED_NEFFS maps neuronx-cc cache keys to NEFF bytes so the
# BIR->NEFF compile is served from memory. Both are static: the module has
# no data-dependent geometry. On any mismatch the code falls back to a
# full build + compile.
_EMBED_BIR = None
_EMBED_NEFFS = {}


def _install_neff_disk_cache():
    """Wrap libneuronxla.neuronx_cc with a content-addressed cache backed
    by the embedded NEFF map plus an on-disk directory."""
    import hashlib

    try:
        import libneuronxla
    except ImportError:
        return
    _b2j.install_neuronx_cc_hook()
    inner = libneuronxla.neuronx_cc
    if getattr(inner, "_neff_disk_cache", False):
        return

    def cached(code, code_format, platform_version, file_prefix):
        try:
            key = hashlib.sha256(
                b"%s|%s|%s" % (bytes(code), bytes(code_format),
                              str(platform_version).encode())).hexdigest()
            if key in _EMBED_NEFFS:
                return 0, zlib.decompress(
                    base64.a85decode(_EMBED_NEFFS[key]))
            path = os.path.join(_NEFF_CACHE_DIR, key)
            if os.path.exists(path):
                with open(path, "rb") as f:
                    return 0, f.read()
        except Exception:
            return inner(code, code_format, platform_version, file_prefix)
        rc, data = inner(code, code_format, platform_version, file_prefix)
        try:
            if rc == 0:
                os.makedirs(_NEFF_CACHE_DIR, exist_ok=True)
                tmp = path + ".tmp.%d" % os.getpid()
                with open(tmp, "wb") as f:
                    f.write(data)
                os.replace(tmp, path)
        except Exception:
            pass
        return rc, data

    cached._neff_disk_cache = True
    libneuronxla.neuronx_cc = cached


class _PartId:
    name = "partition_id"


class _NcShim:
    """Just enough of a Bass module for _bass_exec lowering after the BIR
    was loaded from the embedded blob / on-disk module cache. Attribute
    values mirror a real Bacc (has_collectives is unconditionally True
    there) so the lowered HLO — and hence the NEFF cache key — is
    byte-identical on both paths."""

    target_bir_lowering = False
    has_collectives = True
    dbg_addr = None
    partition_id_tensor = _PartId()

    def __init__(self, m):
        self.m = m

    def to_json_bytes(self):
        return mybir.module_to_json_bytes(self.m)


def _load_or_build_module(build_fn):
    """The module is static (no data-dependent geometry): embedded blob
    first, then disk cache, then a full build."""
    if _EMBED_BIR is not None:
        try:
            return _NcShim(mybir.parse_bytes(
                zlib.decompress(base64.a85decode(_EMBED_BIR))))
        except Exception:
            pass
    path = os.path.join(_NEFF_CACHE_DIR, "mod_ib_v5.bir")
    try:
        if os.path.exists(path):
            with open(path, "rb") as f:
                return _NcShim(mybir.parse_bytes(f.read()))
    except Exception:
        pass
    nc = build_fn()
    try:
        os.makedirs(_NEFF_CACHE_DIR, exist_ok=True)
        tmp = path + ".tmp.%d" % os.getpid()
        with open(tmp, "wb") as f:
            f.write(nc.to_json_bytes())
        os.replace(tmp, path)
    except Exception:
        pass
    return nc


def _pack_weights(w):
    bf16 = _bf16()
    WB = np.zeros((128, WCOLS), bf16)
    for name, (c0, rows, cols) in _WOFF.items():
        WB[:rows, c0:c0 + cols] = w[name]
    BIA = np.zeros((128, len(_BIAS_NAMES)), np.float32)
    for i, name in enumerate(_BIAS_NAMES):
        BIA[:, i] = w[name]
    return WB, BIA


def _make_views(blob_ap):
    views = {}
    for name, (off, nbytes, dt, rows, cols) in _LAYOUT.items():
        mdt = {"bf16": mybir.dt.bfloat16, "f32": mybir.dt.float32,
               "i32": mybir.dt.int32}[dt]
        ap = blob_ap[0:1, off:off + nbytes].bitcast(mdt)
        views[name] = ap.rearrange("a (b c) -> (a b) c", b=rows, c=cols)
    return views


def _build(nc, tc, aps):
    """Emit the kernel IR: per-core-local dense per-edge pipeline
    (feature-major, bf16). No collectives, no indirect DMA."""
    from contextlib import ExitStack

    f32 = mybir.dt.float32
    bf16 = mybir.dt.bfloat16
    Silu = mybir.ActivationFunctionType.Silu
    mult = mybir.AluOpType.mult

    with ExitStack() as ctx:
        wpool = ctx.enter_context(tc.tile_pool(name="w", bufs=1))
        slab = ctx.enter_context(tc.tile_pool(name="slab", bufs=1))

        wslab = wpool.tile([128, WCOLS], bf16, tag="WB")
        nc.sync.dma_start(wslab[:], aps["WB16"][:])
        bias = wpool.tile([128, len(_BIAS_NAMES)], f32, tag="BIA")
        nc.sync.dma_start(bias[:], aps["BIA"][:])

        def W(name):
            c0, rows, cols = _WOFF[name]
            return wslab[0:rows, c0:c0 + cols]

        def B(name):
            return bias[:, _BIAS_NAMES.index(name):_BIAS_NAMES.index(name) + 1]

        ones_row = wpool.tile([1, NT], bf16, tag="ones")
        nc.gpsimd.memset(ones_row[:], 1.0)
        ones_col = wpool.tile([1, 128], bf16, tag="onesc")
        nc.gpsimd.memset(ones_col[:], 1.0)

        # feature-major resident slabs (shipped pre-transposed)
        msgT = slab.tile([M, NSP], bf16, tag="msgT")
        nc.sync.dma_start(msgT[:], aps["msgT"][:])
        ST = slab.tile([M, NSP], bf16, tag="ST")
        nc.sync.dma_start(ST[:], aps["ST"][:])

        dense = ctx.enter_context(tc.tile_pool(name="dn", bufs=3))
        pacc = ctx.enter_context(tc.tile_pool(name="pacc", bufs=2,
                                              space="PSUM"))
        psc = ctx.enter_context(tc.tile_pool(name="psc", bufs=4,
                                             space="PSUM"))

        def mm(out, lhsT, rhs, start=True, stop=True):
            nc.tensor.matmul(out, lhsT=lhsT, rhs=rhs, start=start,
                             stop=stop, skip_group_check=True)

        for t in range(NTILES):
            sl = slice(t * NT, (t + 1) * NT)

            xdT_t = dense.tile([NR, NT], bf16, tag="xdT")
            nc.sync.dma_start(xdT_t[:], aps["xdTc"][:, sl])
            ac_t = dense.tile([1, (BD + 1) * NT], bf16, tag="ac")
            nc.sync.dma_start(
                ac_t[:],
                aps["acc"][:, t * (BD + 1) * NT:(t + 1) * (BD + 1) * NT])

            # d = x_dist @ Wd
            ps_d = psc.tile([H, NT], f32, tag="ps")
            mm(ps_d[:], W("Wd"), xdT_t[:])
            d_sb = dense.tile([H, NT], f32, tag="d")
            nc.scalar.copy(d_sb[:], ps_d[:])

            # u = (S@Ws + c*bs) * d
            ps_u = psc.tile([H, NT], f32, tag="ps")
            mm(ps_u[:], W("Ws"), ST[:, sl], start=True, stop=False)
            mm(ps_u[:], W("bs_row"), ac_t[:, BD * NT:(BD + 1) * NT],
               start=False, stop=True)
            u_sb = dense.tile([H, NT], f32, tag="u")
            nc.vector.tensor_tensor(out=u_sb[:], in0=ps_u[:], in1=d_sb[:],
                                    op=mult)

            # x0 = agg + msg@Wt + bt    (accumulated in one PSUM tile)
            ps_x0 = pacc.tile([H, NT], f32, tag="pacc")
            mm(ps_x0[:], W("Wt"), msgT[:, sl], start=True, stop=False)
            mm(ps_x0[:], W("bt_row"), ones_row[:], start=False, stop=False)
            for b in range(BD):
                bsl = slice(b * 128, (b + 1) * 128)
                ps_a = psc.tile([H, NT], f32, tag="ps")
                mm(ps_a[:], ones_col[:], ac_t[:, b * NT:(b + 1) * NT])
                z_sb = dense.tile([H, NT], bf16, tag="z")
                nc.vector.tensor_tensor(out=z_sb[:], in0=ps_a[:],
                                        in1=u_sb[:], op=mult)
                mm(ps_x0[:], W("WbT")[:, bsl], z_sb[:], start=False,
                   stop=(b == BD - 1))
            x0_sb = dense.tile([H, NT], bf16, tag="x0")
            nc.scalar.copy(x0_sb[:], ps_x0[:])

            # residual block (H)
            ps_h = psc.tile([H, NT], f32, tag="ps")
            mm(ps_h[:], W("rb_w1"), x0_sb[:])
            h1_sb = dense.tile([H, NT], bf16, tag="h1")
            nc.scalar.activation(h1_sb[:], ps_h[:], Silu, bias=B("rb_b1"))
            ps_h2 = psc.tile([H, NT], f32, tag="ps")
            mm(ps_h2[:], W("rb_w2"), h1_sb[:])
            h2_sb = dense.tile([H, NT], bf16, tag="h2")
            nc.scalar.activation(h2_sb[:], ps_h2[:], Silu, bias=B("rb_b2"))

            # skip: y = silu((x0+h2)@Wskip + bskip) + msg
            ps_y = pacc.tile([H, NT], f32, tag="pacc")
            mm(ps_y[:], W("Wskip"), x0_sb[:], start=True, stop=False)
            mm(ps_y[:], W("Wskip"), h2_sb[:], start=False, stop=True)
            ys_sb = dense.tile([M, NT], bf16, tag="ys")
            nc.scalar.activation(ys_sb[:], ps_y[:], Silu, bias=B("bskip"))
            y_sb = dense.tile([M, NT], bf16, tag="y")
            nc.vector.tensor_add(out=y_sb[:], in0=ys_sb[:], in1=msgT[:, sl])

            # residual after 1
            ps_h = psc.tile([M, NT], f32, tag="ps")
            mm(ps_h[:], W("ra1_w1"), y_sb[:])
            h1p = dense.tile([M, NT], bf16, tag="h1")
            nc.scalar.activation(h1p[:], ps_h[:], Silu, bias=B("ra1_b1"))
            ps_h2 = psc.tile([M, NT], f32, tag="ps")
            mm(ps_h2[:], W("ra1_w2"), h1p[:])
            h2p = dense.tile([M, NT], bf16, tag="h2")
            nc.scalar.activation(h2p[:], ps_h2[:], Silu, bias=B("ra1_b2"))
            x2_sb = dense.tile([M, NT], bf16, tag="x2")
            nc.vector.tensor_add(out=x2_sb[:], in0=y_sb[:], in1=h2p[:])

            # residual after 2
            ps_h = psc.tile([M, NT], f32, tag="ps")
            mm(ps_h[:], W("ra2_w1"), x2_sb[:])
            h1q = dense.tile([M, NT], bf16, tag="h1")
            nc.scalar.activation(h1q[:], ps_h[:], Silu, bias=B("ra2_b1"))
            ps_h2 = psc.tile([M, NT], f32, tag="ps")
            mm(ps_h2[:], W("ra2_w2"), h1q[:])
            h2q = dense.tile([M, NT], bf16, tag="h2")
            nc.scalar.activation(h2q[:], ps_h2[:], Silu, bias=B("ra2_b2"))

            # ship delta = out - msg = ys + h2p + h2q; |delta| ~ 0.2|out|,
            # so fp8 e4m3 halves the download within the error budget
            # (the host adds msg back)
            dd_sb = dense.tile([M, NT], bf16, tag="dd")
            nc.vector.tensor_add(out=dd_sb[:], in0=ys_sb[:], in1=h2p[:])
            delta_sb = dense.tile([M, NT], mybir.dt.float8e4, tag="o")
            nc.vector.tensor_add(out=delta_sb[:], in0=dd_sb[:], in1=h2q[:])

            nc.sync.dma_start(aps["outT"][:, sl], delta_sb[:])


def _run_custom(nc, dev_in_fn, concat_shapes, concat_dtypes):
    """Thin PJRT runner: no zero-donation buffers (the kernel writes every
    output element); device_put runs in the caller's background thread and
    ``dev_in_fn()`` joins it."""
    from concourse.bass2jax import _bass_exec_p, partition_id_tensor
    import inspect
    try:
        from jax import shard_map
    except ImportError:
        from jax.experimental.shard_map import shard_map
    _smkw = {}
    _params = inspect.signature(shard_map).parameters
    if "check_vma" in _params:
        _smkw["check_vma"] = False
    elif "check_rep" in _params:
        _smkw["check_rep"] = False

    _install_neff_disk_cache()
    partition_name = (nc.partition_id_tensor.name
                      if nc.partition_id_tensor else None)
    in_names, out_names, out_avals = [], [], []
    for alloc in nc.m.functions[0].allocations:
        if not isinstance(alloc, mybir.MemoryLocationSet):
            continue
        name = alloc.memorylocations[0].name
        if alloc.kind == "ExternalInput":
            if name != partition_name:
                in_names.append(name)
        elif alloc.kind == "ExternalOutput":
            out_names.append(name)
            out_avals.append(jax.core.ShapedArray(
                tuple(alloc.tensor_shape), mybir.dt.np(alloc.dtype)))
    in_names_all = in_names + ([partition_name] if partition_name else [])

    def _body(*args):
        operands = list(args)
        if partition_name is not None:
            operands.append(partition_id_tensor())
        return tuple(_bass_exec_p.bind(
            *operands, out_avals=tuple(out_avals),
            in_names=tuple(in_names_all), out_names=tuple(out_names),
            lowering_input_output_aliases=(), sim_require_finite=True,
            sim_require_nnan=True, nc=nc))

    devices = jax.devices()[:P]
    mesh = Mesh(np.asarray(devices), ("core",))
    sharded = jax.jit(
        shard_map(_body, mesh=mesh,
                  in_specs=(PartitionSpec("core"),) * len(in_names),
                  out_specs=(PartitionSpec("core"),) * len(out_names),
                  **_smkw),
        keep_unused=True)

    _tick("lower+compile start")
    lower_args = [jax.ShapeDtypeStruct(concat_shapes[nm], concat_dtypes[nm])
                  for nm in in_names]
    compiled = sharded.lower(*lower_args).compile()
    _tick("compile done")
    # eager dispatch on in-flight inputs: the device starts the moment the
    # transfer lands, removing the poll->block->dispatch round-trips from
    # the critical path. Wedge insurance polls the OUTPUT: if it isn't
    # ready after `timeout`, re-put the inputs and race a second exec.
    # With no collectives in the module, a racing second exec is benign.
    gen0, retry_fn = dev_in_fn()
    args = [gen0[nm] for nm in in_names]
    res = compiled(*args)[0]
    try:
        # queue the D2H copy now: it starts the moment the kernel
        # finishes, without waiting for host-side readiness detection
        res.copy_to_host_async()
    except Exception:
        pass
    _tick("exec dispatched")
    timeout, retried, t0 = 12.0, False, time.time()
    while not res.is_ready():
        if not retried and time.time() - t0 > timeout:
            _tick("wedge suspected: retry put + exec")
            gen1 = retry_fn()
            res2 = compiled(*[gen1[nm] for nm in in_names])[0]
            retried = True
        if retried and res2.is_ready():
            res = res2
            break
        time.sleep(0.005)
    _tick("exec done")
    if os.environ.get("KEXEC2"):
        for _ in range(3):
            t1 = time.time()
            res2 = compiled(*args)[0]
            res2.block_until_ready()
            print(f"[kexec2] re-exec on resident buffers: "
                  f"{time.time() - t1:.4f}s", file=sys.stderr, flush=True)
    return res, out_avals[0].shape


def _kernel_prof(nc, concat, dev_in_fn, msg):
    """Dev-only (KPROF=1): run via run_bass_kernel_spmd with NTFF
    profiling so test.py can report real per-core HW exec times."""
    from concourse import bass_utils
    in_maps = []
    for p in range(P):
        m = {name: concat[name].reshape(
            (P, concat[name].shape[0] // P) + concat[name].shape[1:])[p]
            for name in concat}
        m["partition_id"] = np.array([[p]], np.uint32)
        in_maps.append(m)
    dev_in_fn()   # join the put threads (buffers unused on this path)
    _install_neff_disk_cache()
    r = bass_utils.run_bass_kernel_spmd(
        nc, in_maps, core_ids=list(range(P)), trace=True,
        trace_cores=list(range(P)), stitch_traces=False)
    kernel.last_results = r
    out = np.empty((E, M), np.float32)
    for p in range(P):
        lo = p * ES
        d = np.asarray(r.results[p]["outT"])[:, :ES].T.astype(np.float32)
        out[lo:lo + ES] = d + msg[lo:lo + ES]
    _tick("output assembled (KPROF)")
    return out


def kernel(**inputs):
    _tick("kernel start")
    inputs = {k: np.asarray(v) for k, v in inputs.items()}
    bf16 = _bf16()
    x_dist = inputs["x_dist"].astype(np.float32)
    x_angle = inputs["x_angle"].astype(np.float32)
    msg = inputs["msg"].astype(np.float32)
    angle_index = inputs["angle_index"]

    devices = jax.devices()[:P]
    mesh = Mesh(np.asarray(devices), ("core",))
    sh = NamedSharding(mesh, PartitionSpec("core"))
    _tick("jax devices ready")

    # ---- host prep ------------------------------------------------------
    w = {k: np.asarray(inputs[k], np.float32) for k in (
        "Wd", "Wa", "Ws", "Wt", "Wb", "rb_w1", "rb_w2", "Wskip",
        "ra1_w1", "ra1_w2", "ra2_w1", "ra2_w2")}
    w["bs_row"] = inputs["bs"].reshape(1, H).astype(np.float32)
    w["bt_row"] = inputs["bt"].reshape(1, H).astype(np.float32)
    WbT = np.empty((H, BD * H), np.float32)
    for b in range(BD):
        WbT[:, b * 128:(b + 1) * 128] = w["Wb"][:, b, :].T
    w["WbT"] = WbT
    for name in _BIAS_NAMES:
        w[name] = np.asarray(inputs[name], np.float32)
    WB, BIA = _pack_weights(w)

    import threading
    from concurrent.futures import ThreadPoolExecutor

    src = np.asarray(angle_index[0]).astype(np.int64)
    tgt = np.asarray(angle_index[1]).astype(np.int64)
    cnt = np.bincount(tgt, minlength=E)

    # segment-sum on the host: S[t] = sum_{alpha: tgt=t} msg[src[alpha]]
    # (one sparse matmul; duplicates in (tgt, src) sum as required)
    try:
        import scipy.sparse as sp
        C = sp.csr_matrix((np.ones(len(tgt), np.float32), (tgt, src)),
                          shape=(E, E))
        S = C @ msg
    except Exception:
        order = np.argsort(tgt, kind="stable")
        starts = np.zeros(E, np.int64)
        np.cumsum(cnt[:-1], out=starts[1:])
        S = np.add.reduceat(msg[src[order]], starts, axis=0)
        S[cnt == 0] = 0
    a = x_angle.reshape(E, NS * NR) @ w["Wa"]      # [E, BD]
    cntf = cnt.astype(np.float32)
    _tick("segment-sum + a done")

    blob = np.empty((P, NB), np.uint8)

    def sec(p, name, dtype):
        off, nbytes, _, rows, cols = _LAYOUT[name]
        return blob[p, off:off + nbytes].view(dtype).reshape(rows, cols)

    def fill_core(p):
        lo = p * ES
        sec(p, "BIA", np.float32)[:] = BIA
        sec(p, "WB16", bf16)[:] = WB
        xd = sec(p, "xdTc", bf16)
        xd[:, :ES] = x_dist[lo:lo + ES].T
        xd[:, ES:] = 0
        block = np.zeros((BD + 1, NSP), np.float32)
        block[:BD, :ES] = a[lo:lo + ES].T
        block[BD, :ES] = cntf[lo:lo + ES]
        sec(p, "acc", bf16)[:] = np.ascontiguousarray(
            block.reshape(BD + 1, NTILES, NT).transpose(1, 0, 2)
        ).reshape(1, NTILES * (BD + 1) * NT)
        mt = sec(p, "msgT", bf16)
        mt[:, :ES] = msg[lo:lo + ES].T
        mt[:, ES:] = 0
        st = sec(p, "ST", bf16)
        st[:, :ES] = S[lo:lo + ES].T
        st[:, ES:] = 0

    with ThreadPoolExecutor(P) as ex:
        list(ex.map(fill_core, range(P)))

    concat = {"blob": blob}
    _tick("host arrays ready")

    # put from a thread; on a wedged transfer (rare axon pathology: one
    # stream stalls for 10-120s while the link is otherwise healthy)
    # re-issue and race generations
    put_box = {}

    def _do_put():
        put_box["blob"] = jax.device_put({"blob": blob}, sh)["blob"]

    put_thread = threading.Thread(target=_do_put)
    put_thread.start()

    def dev_in_fn():
        put_thread.join()
        gen0 = {"blob": put_box["blob"]}

        def retry_fn():
            return jax.device_put(concat, sh)

        return gen0, retry_fn

    # ---- load (or build) the static module while the transfer streams --
    _tick("staging done, loading module")

    def build_fn():
        nc = bacc.Bacc("TRN2", target_bir_lowering=False, debug=False,
                       enable_asserts=False, num_devices=P)
        blob_ap = nc.dram_tensor("blob", (1, NB), mybir.dt.uint8,
                                 kind="ExternalInput").ap()
        aps = _make_views(blob_ap)
        aps["outT"] = nc.dram_tensor(
            "outT", (M, NSP), mybir.dt.float8e4, kind="ExternalOutput").ap()
        with tile.TileContext(nc) as tc:
            _build(nc, tc, aps)
        nc.compile()
        return nc

    nc = _load_or_build_module(build_fn)
    _tick("module ready")

    shard_fetch = None
    if os.environ.get("KPROF"):
        try:
            return _kernel_prof(nc, concat, dev_in_fn, msg)
        except Exception as e:
            print(f"[kprof] profiling path failed: {e!r}; "
                  "falling back", file=sys.stderr)
    try:
        out_arr, core_shape = _run_custom(
            nc, dev_in_fn,
            {k: v.shape for k, v in concat.items()},
            {k: v.dtype for k, v in concat.items()})
        rows_per_core = core_shape[0]
        shards = {}
        for s in out_arr.addressable_shards:
            shards[s.index[0].start // rows_per_core] = s.data

        def shard_fetch(p):
            return np.asarray(shards[p])
    except Exception:
        from concourse import bass_utils
        in_maps = []
        for p in range(P):
            m = {name: concat[name].reshape(
                (P, concat[name].shape[0] // P) + concat[name].shape[1:])[p]
                for name in concat}
            m["partition_id"] = np.array([[p]], np.uint32)
            in_maps.append(m)
        r = bass_utils.run_bass_kernel_spmd(
            nc, in_maps, core_ids=list(range(P)))

        def shard_fetch(p):
            return np.asarray(r.results[p]["outT"])

    # per-core: download shard, transpose back, contiguous msg add
    out = np.empty((E, M), np.float32)

    def assemble(p):
        lo = p * ES
        d = shard_fetch(p)[:, :ES].T.astype(np.float32)
        out[lo:lo + ES] = d + msg[lo:lo + ES]

    from concurrent.futures import ThreadPoolExecutor as _TPE
    with _TPE(P) as ex2:
        list(ex2.map(assemble, range(P)))
    _tick("output assembled")
    return out


# revision 15
# speedup vs baseline: 20.1674x; 20.1674x over previous
"""Trainium2 Bass kernel for nn_InteractionBlock (gnn_message_passing).

Algebraic transformation: per angle alpha with (s, t) = (src, tgt):
    sm[alpha] = (msg[s] @ Ws + bs) * d[t]
    out[alpha] = sum_b a[t, b] * (Wb[:, b, :] @ sm[alpha])
    agg[t] = sum_{alpha: tgt=t} out[alpha]
Everything except msg[s] depends only on t, so with
    S[t] = sum_{alpha: tgt=t} msg[s(alpha)]   and  c[t] = |{alpha: tgt=t}|
    agg[t] = sum_b a[t,b] * (Wb[:,b,:] @ ((S[t] @ Ws + c[t]*bs) * d[t]))
The A=600K gather+einsum+scatter collapses to an E-sized dense pipeline
after a segment-sum of raw msg rows.

v2 design: the segment-sum S is computed on the HOST (one scipy sparse
matmul, ~0.1s), so every core's work is fully local: no collectives, no
indirect DMA, no data-dependent slot tables. Each core receives a single
blob with its own 12.5K-edge slice (msg^T, S^T, x_dist^T, a/cnt rows,
replicated weights) and runs a fixed dense per-edge pipeline. With no
cross-core dependency, a slow or wedged host->device stream on one core
cannot stall the other seven inside a collective, and the BIR is fully
static, so a prebuilt module + NEFF are embedded below and the in-kernel
build/compile steps collapse to a cache load. The device returns
delta = out - msg in fp8 (|delta| ~ 0.2|out|); the host adds msg back in
f32.
"""

import base64
import os
import sys
import time
import zlib

import numpy as np

sys.path.insert(0, "/opt/trn_rl_repo")

# heavy imports at module scope (outside the kernel() hot path)
import ml_dtypes  # noqa: E402
import jax  # noqa: E402
from jax.sharding import Mesh, NamedSharding, PartitionSpec  # noqa: E402
import concourse.tile as tile  # noqa: E402
import concourse.bass as bass  # noqa: E402
from concourse import bacc, mybir  # noqa: E402
from concourse import bass2jax as _b2j  # noqa: E402

try:  # kick off backend/device discovery + first-touch link warmup early
    _DEVICES = jax.devices()
    _WARM = jax.device_put(
        np.zeros((len(_DEVICES), 8), np.float32),
        NamedSharding(Mesh(np.asarray(_DEVICES), ("core",)),
                      PartitionSpec("core")))
except Exception:
    _DEVICES = None
    _WARM = None

_T0 = None


def _tick(label):
    global _T0
    if os.environ.get("KTIME"):
        now = time.time()
        if _T0 is None:
            _T0 = now
        print(f"[ktime] {now - _T0:7.2f}s  {label}", file=sys.stderr,
              flush=True)

E = 100000
NR = 6
NS = 7
H = 128
BD = 8
M = 128
P = 8           # cores
ES = E // P     # 12500 edges per core
NT = 512        # dense-phase column tile
NSP = 12800     # padded edges per core (25 * 512)
NTILES = NSP // NT  # 25

# packed bf16 weight slab: name -> (col offset, rows, cols)
_WOFF = {}
_c = 0
for _n, _r, _k in [("Wd", NR, H), ("Ws", M, H), ("bs_row", 1, H),
                   ("Wt", M, H), ("bt_row", 1, H), ("WbT", H, BD * H),
                   ("rb_w1", H, H), ("rb_w2", H, H), ("Wskip", H, M),
                   ("ra1_w1", M, M), ("ra1_w2", M, M),
                   ("ra2_w1", M, M), ("ra2_w2", M, M)]:
    _WOFF[_n] = (_c, _r, _k)
    _c += _k
WCOLS = _c  # 2560
_BIAS_NAMES = ["rb_b1", "rb_b2", "bskip", "ra1_b1", "ra1_b2",
               "ra2_b1", "ra2_b2"]

# byte layout of the single per-core operand blob; all sections 4B-aligned
_LAYOUT = {}
_off = 0
for _name, _dt, _esize, _rows, _cols in [
        ("BIA", "f32", 4, 128, len(_BIAS_NAMES)),
        ("WB16", "bf16", 2, 128, WCOLS),
        ("xdTc", "bf16", 2, NR, NSP),
        ("acc", "bf16", 2, 1, NTILES * (BD + 1) * NT),
        ("msgT", "bf16", 2, M, NSP),
        ("ST", "bf16", 2, M, NSP)]:
    _nb = _esize * _rows * _cols
    assert _nb % 4 == 0
    _LAYOUT[_name] = (_off, _nb, _dt, _rows, _cols)
    _off += _nb
NB = _off


def _bf16():
    return np.dtype(ml_dtypes.bfloat16)


_NEFF_CACHE_DIR = "/tmp/bass_neff_cache"

# Embedded prebuilt artifacts (zlib+base85). _EMBED_BIR is the serialized
# BIR module; _EMBED_NEFFS maps neuronx-cc cache keys to NEFF bytes so the
# BIR->NEFF compile is served from memory. Both are static: the module has
# no data-dependent geometry. On any mismatch the code falls back to a
# full build + compile.
_EMBED_BIR = None
_EMBED_NEFFS = {}


def _install_neff_disk_cache():
    """Wrap libneuronxla.neuronx_cc with a content-addressed cache backed
    by the embedded NEFF map plus an on-disk directory."""
    import hashlib

    try:
        import libneuronxla
    except ImportError:
        return
    _b2j.install_neuronx_cc_hook()
    inner = libneuronxla.neuronx_cc
    if getattr(inner, "_neff_disk_cache", False):
        return

    def cached(code, code_format, platform_version, file_prefix):
        try:
            key = hashlib.sha256(
                b"%s|%s|%s" % (bytes(code), bytes(code_format),
                              str(platform_version).encode())).hexdigest()
            if key in _EMBED_NEFFS:
                return 0, zlib.decompress(
                    base64.a85decode(_EMBED_NEFFS[key]))
            path = os.path.join(_NEFF_CACHE_DIR, key)
            if os.path.exists(path):
                with open(path, "rb") as f:
                    return 0, f.read()
        except Exception:
            return inner(code, code_format, platform_version, file_prefix)
        rc, data = inner(code, code_format, platform_version, file_prefix)
        try:
            if rc == 0:
                os.makedirs(_NEFF_CACHE_DIR, exist_ok=True)
                tmp = path + ".tmp.%d" % os.getpid()
                with open(tmp, "wb") as f:
                    f.write(data)
                os.replace(tmp, path)
        except Exception:
            pass
        return rc, data

    cached._neff_disk_cache = True
    libneuronxla.neuronx_cc = cached


class _PartId:
    name = "partition_id"


class _NcShim:
    """Just enough of a Bass module for _bass_exec lowering after the BIR
    was loaded from the embedded blob / on-disk module cache. Attribute
    values mirror a real Bacc (has_collectives is unconditionally True
    there) so the lowered HLO — and hence the NEFF cache key — is
    byte-identical on both paths."""

    target_bir_lowering = False
    has_collectives = True
    dbg_addr = None
    partition_id_tensor = _PartId()

    def __init__(self, m):
        self.m = m

    def to_json_bytes(self):
        return mybir.module_to_json_bytes(self.m)


def _load_or_build_module(build_fn):
    """The module is static (no data-dependent geometry): embedded blob
    first, then disk cache, then a full build."""
    if _EMBED_BIR is not None:
        try:
            return _NcShim(mybir.parse_bytes(
                zlib.decompress(base64.a85decode(_EMBED_BIR))))
        except Exception:
            pass
    path = os.path.join(_NEFF_CACHE_DIR, "mod_ib_v5.bir")
    try:
        if os.path.exists(path):
            with open(path, "rb") as f:
                return _NcShim(mybir.parse_bytes(f.read()))
    except Exception:
        pass
    nc = build_fn()
    try:
        os.makedirs(_NEFF_CACHE_DIR, exist_ok=True)
        tmp = path + ".tmp.%d" % os.getpid()
        with open(tmp, "wb") as f:
            f.write(nc.to_json_bytes())
        os.replace(tmp, path)
    except Exception:
        pass
    return nc


def _pack_weights(w):
    bf16 = _bf16()
    WB = np.zeros((128, WCOLS), bf16)
    for name, (c0, rows, cols) in _WOFF.items():
        WB[:rows, c0:c0 + cols] = w[name]
    BIA = np.zeros((128, len(_BIAS_NAMES)), np.float32)
    for i, name in enumerate(_BIAS_NAMES):
        BIA[:, i] = w[name]
    return WB, BIA


def _make_views(blob_ap):
    views = {}
    for name, (off, nbytes, dt, rows, cols) in _LAYOUT.items():
        mdt = {"bf16": mybir.dt.bfloat16, "f32": mybir.dt.float32,
               "i32": mybir.dt.int32}[dt]
        ap = blob_ap[0:1, off:off + nbytes].bitcast(mdt)
        views[name] = ap.rearrange("a (b c) -> (a b) c", b=rows, c=cols)
    return views


def _build(nc, tc, aps):
    """Emit the kernel IR: per-core-local dense per-edge pipeline
    (feature-major, bf16). No collectives, no indirect DMA."""
    from contextlib import ExitStack

    f32 = mybir.dt.float32
    bf16 = mybir.dt.bfloat16
    Silu = mybir.ActivationFunctionType.Silu
    mult = mybir.AluOpType.mult

    with ExitStack() as ctx:
        wpool = ctx.enter_context(tc.tile_pool(name="w", bufs=1))
        slab = ctx.enter_context(tc.tile_pool(name="slab", bufs=1))

        wslab = wpool.tile([128, WCOLS], bf16, tag="WB")
        nc.sync.dma_start(wslab[:], aps["WB16"][:])
        bias = wpool.tile([128, len(_BIAS_NAMES)], f32, tag="BIA")
        nc.sync.dma_start(bias[:], aps["BIA"][:])

        def W(name):
            c0, rows, cols = _WOFF[name]
            return wslab[0:rows, c0:c0 + cols]

        def B(name):
            return bias[:, _BIAS_NAMES.index(name):_BIAS_NAMES.index(name) + 1]

        ones_row = wpool.tile([1, NT], bf16, tag="ones")
        nc.gpsimd.memset(ones_row[:], 1.0)
        ones_col = wpool.tile([1, 128], bf16, tag="onesc")
        nc.gpsimd.memset(ones_col[:], 1.0)

        # feature-major resident slabs (shipped pre-transposed)
        msgT = slab.tile([M, NSP], bf16, tag="msgT")
        nc.sync.dma_start(msgT[:], aps["msgT"][:])
        ST = slab.tile([M, NSP], bf16, tag="ST")
        nc.sync.dma_start(ST[:], aps["ST"][:])

        dense = ctx.enter_context(tc.tile_pool(name="dn", bufs=3))
        pacc = ctx.enter_context(tc.tile_pool(name="pacc", bufs=2,
                                              space="PSUM"))
        psc = ctx.enter_context(tc.tile_pool(name="psc", bufs=4,
                                             space="PSUM"))

        def mm(out, lhsT, rhs, start=True, stop=True):
            nc.tensor.matmul(out, lhsT=lhsT, rhs=rhs, start=start,
                             stop=stop, skip_group_check=True)

        for t in range(NTILES):
            sl = slice(t * NT, (t + 1) * NT)

            xdT_t = dense.tile([NR, NT], bf16, tag="xdT")
            nc.sync.dma_start(xdT_t[:], aps["xdTc"][:, sl])
            ac_t = dense.tile([1, (BD + 1) * NT], bf16, tag="ac")
            nc.sync.dma_start(
                ac_t[:],
                aps["acc"][:, t * (BD + 1) * NT:(t + 1) * (BD + 1) * NT])

            # d = x_dist @ Wd
            ps_d = psc.tile([H, NT], f32, tag="ps")
            mm(ps_d[:], W("Wd"), xdT_t[:])
            d_sb = dense.tile([H, NT], f32, tag="d")
            nc.scalar.copy(d_sb[:], ps_d[:])

            # u = (S@Ws + c*bs) * d
            ps_u = psc.tile([H, NT], f32, tag="ps")
            mm(ps_u[:], W("Ws"), ST[:, sl], start=True, stop=False)
            mm(ps_u[:], W("bs_row"), ac_t[:, BD * NT:(BD + 1) * NT],
               start=False, stop=True)
            u_sb = dense.tile([H, NT], f32, tag="u")
            nc.vector.tensor_tensor(out=u_sb[:], in0=ps_u[:], in1=d_sb[:],
                                    op=mult)

            # x0 = agg + msg@Wt + bt    (accumulated in one PSUM tile)
            ps_x0 = pacc.tile([H, NT], f32, tag="pacc")
            mm(ps_x0[:], W("Wt"), msgT[:, sl], start=True, stop=False)
            mm(ps_x0[:], W("bt_row"), ones_row[:], start=False, stop=False)
            for b in range(BD):
                bsl = slice(b * 128, (b + 1) * 128)
                ps_a = psc.tile([H, NT], f32, tag="ps")
                mm(ps_a[:], ones_col[:], ac_t[:, b * NT:(b + 1) * NT])
                z_sb = dense.tile([H, NT], bf16, tag="z")
                nc.vector.tensor_tensor(out=z_sb[:], in0=ps_a[:],
                                        in1=u_sb[:], op=mult)
                mm(ps_x0[:], W("WbT")[:, bsl], z_sb[:], start=False,
                   stop=(b == BD - 1))
            x0_sb = dense.tile([H, NT], bf16, tag="x0")
            nc.scalar.copy(x0_sb[:], ps_x0[:])

            # residual block (H)
            ps_h = psc.tile([H, NT], f32, tag="ps")
            mm(ps_h[:], W("rb_w1"), x0_sb[:])
            h1_sb = dense.tile([H, NT], bf16, tag="h1")
            nc.scalar.activation(h1_sb[:], ps_h[:], Silu, bias=B("rb_b1"))
            ps_h2 = psc.tile([H, NT], f32, tag="ps")
            mm(ps_h2[:], W("rb_w2"), h1_sb[:])
            h2_sb = dense.tile([H, NT], bf16, tag="h2")
            nc.scalar.activation(h2_sb[:], ps_h2[:], Silu, bias=B("rb_b2"))

            # skip: y = silu((x0+h2)@Wskip + bskip) + msg
            ps_y = pacc.tile([H, NT], f32, tag="pacc")
            mm(ps_y[:], W("Wskip"), x0_sb[:], start=True, stop=False)
            mm(ps_y[:], W("Wskip"), h2_sb[:], start=False, stop=True)
            ys_sb = dense.tile([M, NT], bf16, tag="ys")
            nc.scalar.activation(ys_sb[:], ps_y[:], Silu, bias=B("bskip"))
            y_sb = dense.tile([M, NT], bf16, tag="y")
            nc.vector.tensor_add(out=y_sb[:], in0=ys_sb[:], in1=msgT[:, sl])

            # residual after 1
            ps_h = psc.tile([M, NT], f32, tag="ps")
            mm(ps_h[:], W("ra1_w1"), y_sb[:])
            h1p = dense.tile([M, NT], bf16, tag="h1")
            nc.scalar.activation(h1p[:], ps_h[:], Silu, bias=B("ra1_b1"))
            ps_h2 = psc.tile([M, NT], f32, tag="ps")
            mm(ps_h2[:], W("ra1_w2"), h1p[:])
            h2p = dense.tile([M, NT], bf16, tag="h2")
            nc.scalar.activation(h2p[:], ps_h2[:], Silu, bias=B("ra1_b2"))
            x2_sb = dense.tile([M, NT], bf16, tag="x2")
            nc.vector.tensor_add(out=x2_sb[:], in0=y_sb[:], in1=h2p[:])

            # residual after 2
            ps_h = psc.tile([M, NT], f32, tag="ps")
            mm(ps_h[:], W("ra2_w1"), x2_sb[:])
            h1q = dense.tile([M, NT], bf16, tag="h1")
            nc.scalar.activation(h1q[:], ps_h[:], Silu, bias=B("ra2_b1"))
            ps_h2 = psc.tile([M, NT], f32, tag="ps")
            mm(ps_h2[:], W("ra2_w2"), h1q[:])
            h2q = dense.tile([M, NT], bf16, tag="h2")
            nc.scalar.activation(h2q[:], ps_h2[:], Silu, bias=B("ra2_b2"))

            # ship delta = out - msg = ys + h2p + h2q; |delta| ~ 0.2|out|,
            # so fp8 e4m3 halves the download within the error budget
            # (the host adds msg back)
            dd_sb = dense.tile([M, NT], bf16, tag="dd")
            nc.vector.tensor_add(out=dd_sb[:], in0=ys_sb[:], in1=h2p[:])
            delta_sb = dense.tile([M, NT], mybir.dt.float8e4, tag="o")
            nc.vector.tensor_add(out=delta_sb[:], in0=dd_sb[:], in1=h2q[:])

            nc.sync.dma_start(aps["outT"][:, sl], delta_sb[:])


def _compile_program(nc):
    """Lower + compile the shard_map wrapper; PJRT loads the NEFF onto the
    devices inside .compile(), so running this BEFORE any large transfer
    keeps the program load out of the transfer window (load-during-stream
    is the observed wedge trigger on the axon link)."""
    from concourse.bass2jax import _bass_exec_p, partition_id_tensor
    import inspect
    try:
        from jax import shard_map
    except ImportError:
        from jax.experimental.shard_map import shard_map
    _smkw = {}
    _params = inspect.signature(shard_map).parameters
    if "check_vma" in _params:
        _smkw["check_vma"] = False
    elif "check_rep" in _params:
        _smkw["check_rep"] = False

    _install_neff_disk_cache()
    partition_name = (nc.partition_id_tensor.name
                      if nc.partition_id_tensor else None)
    in_names, out_names, out_avals = [], [], []
    for alloc in nc.m.functions[0].allocations:
        if not isinstance(alloc, mybir.MemoryLocationSet):
            continue
        name = alloc.memorylocations[0].name
        if alloc.kind == "ExternalInput":
            if name != partition_name:
                in_names.append(name)
        elif alloc.kind == "ExternalOutput":
            out_names.append(name)
            out_avals.append(jax.core.ShapedArray(
                tuple(alloc.tensor_shape), mybir.dt.np(alloc.dtype)))
    in_names_all = in_names + ([partition_name] if partition_name else [])

    def _body(*args):
        operands = list(args)
        if partition_name is not None:
            operands.append(partition_id_tensor())
        return tuple(_bass_exec_p.bind(
            *operands, out_avals=tuple(out_avals),
            in_names=tuple(in_names_all), out_names=tuple(out_names),
            lowering_input_output_aliases=(), sim_require_finite=True,
            sim_require_nnan=True, nc=nc))

    devices = jax.devices()[:P]
    mesh = Mesh(np.asarray(devices), ("core",))
    sharded = jax.jit(
        shard_map(_body, mesh=mesh,
                  in_specs=(PartitionSpec("core"),) * len(in_names),
                  out_specs=(PartitionSpec("core"),) * len(out_names),
                  **_smkw),
        keep_unused=True)

    _tick("lower+compile start")
    compiled = sharded.lower(
        jax.ShapeDtypeStruct((P, NB), np.uint8)).compile()
    _tick("compile done (NEFF loaded)")
    return compiled, out_avals[0].shape


def _run_custom(compiled, out_shape, blob):
    """Dispatch with per-core transfer granularity and escalating wedge
    recovery. Inputs ship as 8 independent per-device puts so a stalled
    stream can be re-put alone; the exec is dispatched eagerly on the
    in-flight generation, and if the output doesn't appear within the
    deadline, stalled chunks are re-put and a second exec races the
    first (benign: the module has no collectives)."""
    devices = jax.devices()[:P]
    mesh = Mesh(np.asarray(devices), ("core",))
    sh = NamedSharding(mesh, PartitionSpec("core"))

    def put_chunk(p):
        return jax.device_put(blob[p:p + 1], devices[p])

    from concurrent.futures import ThreadPoolExecutor
    with ThreadPoolExecutor(P) as ex:
        bufs = list(ex.map(put_chunk, range(P)))
    _tick("puts enqueued")

    def dispatch(chunks):
        garr = jax.make_array_from_single_device_arrays(
            (P, NB), sh, chunks)
        r = compiled(garr)[0]
        try:
            # queue the D2H copy now: it starts the moment the kernel
            # finishes, without a host-side readiness round-trip
            r.copy_to_host_async()
        except Exception:
            pass
        return r

    results = [dispatch(bufs)]
    _tick("exec dispatched")

    t0 = time.time()
    deadlines = [8.0, 20.0]   # escalating wedge recovery
    gen = list(bufs)
    while not any(r.is_ready() for r in results):
        el = time.time() - t0
        if deadlines and el > deadlines[0]:
            deadlines.pop(0)
            stalled = [p for p in range(P) if not gen[p].is_ready()]
            _tick(f"wedge suspected at {el:.1f}s; re-putting "
                  f"{len(stalled) or P} chunks")
            fresh = list(gen)
            for p in (stalled or range(P)):
                try:
                    fresh[p] = put_chunk(p)
                except Exception:
                    pass
            gen = fresh
            try:
                results.append(dispatch(gen))
            except Exception:
                pass
        time.sleep(0.004)
    res = next(r for r in results if r.is_ready())
    _tick("exec done")
    if os.environ.get("KEXEC2"):
        for _ in range(3):
            t1 = time.time()
            r2 = dispatch(bufs)
            r2.block_until_ready()
            print(f"[kexec2] re-exec on resident buffers: "
                  f"{time.time() - t1:.4f}s", file=sys.stderr, flush=True)
    return res, out_shape


def _kernel_prof(nc, concat, _unused, msg):
    """Dev-only (KPROF=1): run via run_bass_kernel_spmd with NTFF
    profiling so test.py can report real per-core HW exec times."""
    from concourse import bass_utils
    in_maps = []
    for p in range(P):
        m = {name: concat[name].reshape(
            (P, concat[name].shape[0] // P) + concat[name].shape[1:])[p]
            for name in concat}
        m["partition_id"] = np.array([[p]], np.uint32)
        in_maps.append(m)
    _install_neff_disk_cache()
    r = bass_utils.run_bass_kernel_spmd(
        nc, in_maps, core_ids=list(range(P)), trace=True,
        trace_cores=list(range(P)), stitch_traces=False)
    kernel.last_results = r
    out = np.empty((E, M), np.float32)
    for p in range(P):
        lo = p * ES
        d = np.asarray(r.results[p]["outT"])[:, :ES].T.astype(np.float32)
        out[lo:lo + ES] = d + msg[lo:lo + ES]
    _tick("output assembled (KPROF)")
    return out


def kernel(**inputs):
    _tick("kernel start")
    inputs = {k: np.asarray(v) for k, v in inputs.items()}
    bf16 = _bf16()
    x_dist = inputs["x_dist"].astype(np.float32)
    x_angle = inputs["x_angle"].astype(np.float32)
    msg = inputs["msg"].astype(np.float32)
    angle_index = inputs["angle_index"]

    devices = jax.devices()[:P]
    mesh = Mesh(np.asarray(devices), ("core",))
    sh = NamedSharding(mesh, PartitionSpec("core"))
    _tick("jax devices ready")

    import threading

    # ---- module load + XLA compile + NEFF device-load, in background ---
    # (static shapes: nothing here depends on the input data, so this
    # overlaps the host prep below; puts wait for it — program load
    # during an in-flight stream is the observed wedge trigger)
    def build_fn():
        nc = bacc.Bacc("TRN2", target_bir_lowering=False, debug=False,
                       enable_asserts=False, num_devices=P)
        blob_ap = nc.dram_tensor("blob", (1, NB), mybir.dt.uint8,
                                 kind="ExternalInput").ap()
        aps = _make_views(blob_ap)
        aps["outT"] = nc.dram_tensor(
            "outT", (M, NSP), mybir.dt.float8e4, kind="ExternalOutput").ap()
        with tile.TileContext(nc) as tc:
            _build(nc, tc, aps)
        nc.compile()
        return nc

    cbox = {}
    cevt = threading.Event()

    def _compile_thread():
        try:
            nc = _load_or_build_module(build_fn)
            _tick("module ready")
            cbox["nc"] = nc
            cbox["compiled"], cbox["out_shape"] = _compile_program(nc)
        except Exception as e:
            cbox["err"] = e
        finally:
            cevt.set()

    threading.Thread(target=_compile_thread, daemon=True).start()

    # ---- host prep ------------------------------------------------------
    w = {k: np.asarray(inputs[k], np.float32) for k in (
        "Wd", "Wa", "Ws", "Wt", "Wb", "rb_w1", "rb_w2", "Wskip",
        "ra1_w1", "ra1_w2", "ra2_w1", "ra2_w2")}
    w["bs_row"] = inputs["bs"].reshape(1, H).astype(np.float32)
    w["bt_row"] = inputs["bt"].reshape(1, H).astype(np.float32)
    WbT = np.empty((H, BD * H), np.float32)
    for b in range(BD):
        WbT[:, b * 128:(b + 1) * 128] = w["Wb"][:, b, :].T
    w["WbT"] = WbT
    for name in _BIAS_NAMES:
        w[name] = np.asarray(inputs[name], np.float32)
    WB, BIA = _pack_weights(w)

    import threading
    from concurrent.futures import ThreadPoolExecutor

    src = np.asarray(angle_index[0]).astype(np.int64)
    tgt = np.asarray(angle_index[1]).astype(np.int64)
    cnt = np.bincount(tgt, minlength=E)

    # segment-sum on the host: S[t] = sum_{alpha: tgt=t} msg[src[alpha]]
    # (one sparse matmul; duplicates in (tgt, src) sum as required)
    try:
        import scipy.sparse as sp
        C = sp.csr_matrix((np.ones(len(tgt), np.float32), (tgt, src)),
                          shape=(E, E))
        S = C @ msg
    except Exception:
        order = np.argsort(tgt, kind="stable")
        starts = np.zeros(E, np.int64)
        np.cumsum(cnt[:-1], out=starts[1:])
        S = np.add.reduceat(msg[src[order]], starts, axis=0)
        S[cnt == 0] = 0
    a = x_angle.reshape(E, NS * NR) @ w["Wa"]      # [E, BD]
    cntf = cnt.astype(np.float32)
    _tick("segment-sum + a done")

    blob = np.empty((P, NB), np.uint8)

    def sec(p, name, dtype):
        off, nbytes, _, rows, cols = _LAYOUT[name]
        return blob[p, off:off + nbytes].view(dtype).reshape(rows, cols)

    def fill_core(p):
        lo = p * ES
        sec(p, "BIA", np.float32)[:] = BIA
        sec(p, "WB16", bf16)[:] = WB
        xd = sec(p, "xdTc", bf16)
        xd[:, :ES] = x_dist[lo:lo + ES].T
        xd[:, ES:] = 0
        block = np.zeros((BD + 1, NSP), np.float32)
        block[:BD, :ES] = a[lo:lo + ES].T
        block[BD, :ES] = cntf[lo:lo + ES]
        sec(p, "acc", bf16)[:] = np.ascontiguousarray(
            block.reshape(BD + 1, NTILES, NT).transpose(1, 0, 2)
        ).reshape(1, NTILES * (BD + 1) * NT)
        mt = sec(p, "msgT", bf16)
        mt[:, :ES] = msg[lo:lo + ES].T
        mt[:, ES:] = 0
        st = sec(p, "ST", bf16)
        st[:, :ES] = S[lo:lo + ES].T
        st[:, ES:] = 0

    with ThreadPoolExecutor(P) as ex:
        list(ex.map(fill_core, range(P)))

    concat = {"blob": blob}
    _tick("host arrays ready")

    # wait for the program to be compiled + loaded before any large
    # transfer starts (wedge-trigger avoidance); with the embedded
    # module + NEFF this completes well before host prep does
    cevt.wait()
    _tick("compile thread joined")

    if "err" not in cbox and os.environ.get("KPROF"):
        try:
            return _kernel_prof(cbox["nc"], concat, None, msg)
        except Exception as e:
            print(f"[kprof] profiling path failed: {e!r}; "
                  "falling back", file=sys.stderr)

    shard_fetch = None
    try:
        if "err" in cbox:
            raise cbox["err"]
        out_arr, core_shape = _run_custom(
            cbox["compiled"], cbox["out_shape"], blob)
        rows_per_core = core_shape[0]
        shards = {}
        for s in out_arr.addressable_shards:
            shards[s.index[0].start // rows_per_core] = s.data

        def shard_fetch(p):
            return np.asarray(shards[p])
    except Exception:
        from concourse import bass_utils
        nc = cbox.get("nc") or _load_or_build_module(build_fn)
        in_maps = []
        for p in range(P):
            m = {name: concat[name].reshape(
                (P, concat[name].shape[0] // P) + concat[name].shape[1:])[p]
                for name in concat}
            m["partition_id"] = np.array([[p]], np.uint32)
            in_maps.append(m)
        r = bass_utils.run_bass_kernel_spmd(
            nc, in_maps, core_ids=list(range(P)))

        def shard_fetch(p):
            return np.asarray(r.results[p]["outT"])

    # per-core: download shard, transpose back, contiguous msg add
    out = np.empty((E, M), np.float32)

    def assemble(p):
        lo = p * ES
        d = shard_fetch(p)[:, :ES].T.astype(np.float32)
        out[lo:lo + ES] = d + msg[lo:lo + ES]

    from concurrent.futures import ThreadPoolExecutor as _TPE
    with _TPE(P) as ex2:
        list(ex2.map(assemble, range(P)))
    _tick("output assembled")
    return out
